# revision 26
# baseline (speedup 1.0000x reference)
"""AFT (attention-free transformer) block on 8 TRN2 NeuronCores — v8.

Reference computation (T=2048, B=4, D=1024):
    qkv = data @ W_qkv + b_qkv ; q,k,v = split(qkv)
    num = exp(pb - max_pb) @ (exp(k - max_k) * v)    (contraction over key pos j)
    den = exp(pb - max_pb) @ exp(k - max_k)
    out = (sigmoid(q) * num / den) @ W_out + b_out
The max shifts cancel exactly in num/den and value ranges are tiny, so the
kernel drops them.

Sharding: hybrid (sequence-half x batch). Core r = 2b + h owns batch b and
query rows i in [h*1024, (h+1)*1024). Each core projects q/k/v for its own
1024 tokens, the 8 cores all-gather exp(k) and exp(k)*v (bf16, two pipelined
j-half chunks), and each core then reads back ONLY its batch's slice of the
gathered buffer (8MB instead of 32MB) via indirect DMAs whose row indices
are a per-core host input — this keeps the SPMD graph uniform while the
blocks read differ per core.

Pipeline (driven by the ~140us AllGather wire time):
  - k/v columns are projected first (q deferred) so AG chunk 0 triggers early.
  - num/den accumulation is chunk-decoupled: all 8 query-tile groups
    accumulate chunk-0 j-tiles into PSUM and spill bf16 partials to SBUF
    while AG chunk 1 is on the wire; the odd pass adds chunk-1 contributions
    and merges in the epilogue. Keeps the PE busy through the AG window
    despite 8 PSUM banks.
  - every matmul reuses one stationary (lhsT) load for 2-4 N=512 moving
    passes (ldw-opt is off in this compile config, so LDWEIGHTS serialize).
  - y is token-major; [d, i] transposes for the output projection are PE
    transposes; sigmoid(q) needs no shuffling (batch is fixed per core).
"""

import numpy as np
import ml_dtypes

from concourse import bacc, bass, mybir, tile
from concourse.bass_utils import run_bass_kernel_spmd
from concourse.masks import make_identity

BF16 = mybir.dt.bfloat16
F32 = mybir.dt.float32
I32 = mybir.dt.int32
AF = mybir.ActivationFunctionType

N_CORES = 8
T, B, D = 2048, 4, 1024
TOK = 1024                 # tokens per core: 1024 query rows of one batch
KT = D // 128              # 8 contraction tiles for d
NG = TOK // 128            # 8 query-tile groups

_cache = {}


def build(with_qkv_bias: bool, with_out_bias: bool):
    nc = bacc.Bacc(None, target_bir_lowering=False)

    dataT_d = nc.dram_tensor("dataT", [D, TOK], BF16, kind="ExternalInput")
    wkv_d = nc.dram_tensor("wkv", [D, 2 * D], BF16, kind="ExternalInput")
    wq_d = nc.dram_tensor("wq", [D, D], BF16, kind="ExternalInput")
    pbT_d = nc.dram_tensor("pbT", [T, TOK], BF16, kind="ExternalInput")
    wout_d = nc.dram_tensor("wout", [D, D], BF16, kind="ExternalInput")
    gidx_d = nc.dram_tensor("gidx", [128, 32], I32, kind="ExternalInput")
    out_d = nc.dram_tensor("out", [TOK, D], F32, kind="ExternalOutput")
    if with_qkv_bias:
        bkv_d = nc.dram_tensor("bkv", [1, 2 * D], BF16, kind="ExternalInput")
        bq_d = nc.dram_tensor("bq", [1, D], BF16, kind="ExternalInput")
    if with_out_bias:
        bout_d = nc.dram_tensor("bout", [1, D], BF16, kind="ExternalInput")

    with tile.TileContext(nc) as tc:
        with (
            tc.tile_pool(name="persist", bufs=1) as pp,
            tc.tile_pool(name="psum", bufs=6, space="PSUM") as psp,
            tc.tile_pool(name="psum_tr", bufs=2, space="PSUM") as pstr,
            tc.tile_pool(name="dram", bufs=1, space="DRAM") as dram,
        ):
            # ---- persistent SBUF tensors ----
            ident = pp.tile([128, 128], BF16, name="ident", tag="ident")
            make_identity(nc, ident[:])
            gidx = pp.tile([128, 32], I32, name="gidx", tag="gidx")
            nc.sync.dma_start(gidx[:], gidx_d[:])
            wout = [pp.tile([128, D], BF16, name=f"wout{k}", tag=f"wout{k}")
                    for k in range(KT)]
            pbe = [pp.tile([128, TOK], BF16, name=f"pbe{t}", tag=f"pbe{t}")
                   for t in range(T // 128)]
            sq_t = [pp.tile([128, D], BF16, name=f"sq{m}", tag=f"sq{m}")
                    for m in range(NG)]
            if with_qkv_bias or with_out_bias:
                ones1 = pp.tile([1, 128], BF16, name="ones1", tag="ones1")
                nc.gpsimd.memset(ones1[:], 1.0)
            if with_qkv_bias:
                bkv = pp.tile([1, 2 * D], BF16, name="bkv", tag="bkv")
                nc.sync.dma_start(bkv[:], bkv_d[:])
                bq = pp.tile([1, D], BF16, name="bq", tag="bq")
                nc.sync.dma_start(bq[:], bq_d[:])
            if with_out_bias:
                bout = pp.tile([1, D], BF16, name="bout", tag="bout")
                nc.sync.dma_start(bout[:], bout_d[:])

            # collective bounce buffers: two token-half chunks of [ek | ekv]
            cc_in = [dram.tile([TOK, D], BF16, name=f"cc_in{x}") for x in range(2)]
            cc_out = [dram.tile([N_CORES * TOK, D], BF16, name=f"cc_out{x}",
                                addr_space="Shared") for x in range(2)]

            # ---- phase A: qkv projection, k/v first ----
            with tc.tile_pool(name="phaseA", bufs=1) as pa:
                dataT = [pa.tile([128, TOK], BF16, name=f"dataT{k}",
                                 tag=f"dataT{k}") for k in range(KT)]
                wkv = [pa.tile([128, 2 * D], BF16, name=f"wkv{k}",
                               tag=f"wkv{k}") for k in range(KT)]
                wq = [pa.tile([128, D], BF16, name=f"wq{k}", tag=f"wq{k}")
                      for k in range(KT)]
                # kv weights first so pass 1 can start after ~6MB of DMA
                for k in range(KT):
                    nc.sync.dma_start(dataT[k][:], dataT_d[k * 128:(k + 1) * 128, :])
                    nc.sync.dma_start(wkv[k][:], wkv_d[k * 128:(k + 1) * 128, :])
                for k in range(KT):
                    nc.sync.dma_start(wq[k][:], wq_d[k * 128:(k + 1) * 128, :])

                # pass 1: k and v chunks -> exp(k), exp(k)*v -> cc_in -> AG
                for m in range(NG):  # token tile
                    ek = pa.tile([128, D], BF16, name=f"ek{m}", tag="ek", bufs=3)
                    vv = pa.tile([128, D], BF16, name=f"vv{m}", tag="vv", bufs=3)
                    ekv = pa.tile([128, D], BF16, name=f"ekv{m}", tag="ekv", bufs=3)
                    ps = [psp.tile([128, 512], F32, name=f"ps{m}_{i}",
                                   tag="ps") for i in range(4)]
                    for k in range(KT):
                        for i in range(4):
                            nc.tensor.matmul(
                                ps[i][:], dataT[k][:, m * 128:(m + 1) * 128],
                                wkv[k][:, i * 512:(i + 1) * 512],
                                start=(k == 0),
                                stop=(k == KT - 1 and not with_qkv_bias),
                            )
                    if with_qkv_bias:
                        for i in range(4):
                            nc.tensor.matmul(
                                ps[i][:], ones1[:], bkv[:, i * 512:(i + 1) * 512],
                                start=False, stop=True,
                            )
                    for i in range(2):
                        nc.scalar.activation(
                            ek[:, i * 512:(i + 1) * 512], ps[i][:], AF.Exp)
                        nc.vector.tensor_copy(
                            vv[:, i * 512:(i + 1) * 512], ps[2 + i][:])
                    nc.vector.tensor_mul(ekv[:], ek[:], vv[:])
                    # chunk x = m//4 holds token rows [x*512,(x+1)*512):
                    # layout [ek half | ekv half]
                    x, mm = m // 4, m % 4
                    nc.sync.dma_start(
                        cc_in[x][mm * 128:(mm + 1) * 128, :], ek[:])
                    nc.sync.dma_start(
                        cc_in[x][512 + mm * 128:512 + (mm + 1) * 128, :], ekv[:])
                    if m in (3, 7):
                        nc.gpsimd.collective_compute(
                            "AllGather", mybir.AluOpType.bypass,
                            replica_groups=[list(range(N_CORES))],
                            ins=[cc_in[m // 4][:].opt()],
                            outs=[cc_out[m // 4][:].opt()],
                        )

                # exp(pbT) — loads ride behind the cc_in stores, done ~mid-AG
                for t in range(T // 128):
                    praw = pa.tile([128, TOK], BF16, name=f"praw{t}", tag="praw",
                                   bufs=4)
                    nc.sync.dma_start(praw[:], pbT_d[t * 128:(t + 1) * 128, :])
                    nc.scalar.activation(pbe[t][:], praw[:], AF.Exp)

                # pass 2: q chunks + sigmoid — overlaps the collectives
                for m in range(NG):
                    sq = sq_t[m]
                    ps = [psp.tile([128, 512], F32, name=f"psq{m}_{i}",
                                   tag="ps") for i in range(2)]
                    for k in range(KT):
                        for i in range(2):
                            nc.tensor.matmul(
                                ps[i][:], dataT[k][:, m * 128:(m + 1) * 128],
                                wq[k][:, i * 512:(i + 1) * 512],
                                start=(k == 0),
                                stop=(k == KT - 1 and not with_qkv_bias),
                            )
                    if with_qkv_bias:
                        for i in range(2):
                            nc.tensor.matmul(
                                ps[i][:], ones1[:], bq[:, i * 512:(i + 1) * 512],
                                start=False, stop=True,
                            )
                    for i in range(2):
                        nc.scalar.activation(
                            sq[:, i * 512:(i + 1) * 512], ps[i][:], AF.Sigmoid)

                # wout: needed only by the output projection much later
                for k in range(KT):
                    nc.sync.dma_start(wout[k][:], wout_d[k * 128:(k + 1) * 128, :])

            # ---- phase B: num/den (chunk-decoupled) + y + output projection --
            # This core reads only its batch's two rank blocks out of cc_out;
            # the row indices come from gidx (per-core input):
            #   col c = x*16 + half*8 + u' : rows of j-tile u' of chunk x,
            #   half 0 = ek, half 1 = ekv. j-tile u' covers global j tile
            #   t = 8*(u'//4) + 4*x + (u'%4).
            def jtile(x, u):
                return 8 * (u // 4) + 4 * x + (u % 4)

            with tc.tile_pool(name="phaseB", bufs=1) as pbp:
                def load_ekg(x):
                    tiles = []
                    for u in range(N_CORES):
                        g = pbp.tile([128, 2048], BF16, name=f"ekg{x}_{u}",
                                     tag="ekg", bufs=10)
                        for half in range(2):
                            col = x * 16 + half * 8 + u
                            nc.gpsimd.indirect_dma_start(
                                out=g[:, half * D:(half + 1) * D],
                                out_offset=None,
                                in_=cc_out[x][:],
                                in_offset=bass.IndirectOffsetOnAxis(
                                    ap=gidx[:, col:col + 1], axis=0),
                            )
                        tiles.append(g)
                    return tiles

                # even pass: chunk-0 partials for every query-tile group
                spill = {}
                ekg0 = load_ekg(0)
                for m2 in range(NG):
                    pn = [psp.tile([128, 512], F32, name=f"pnE{m2}{i}",
                                   tag="ps") for i in range(2)]
                    pd = [psp.tile([128, 512], F32, name=f"pdE{m2}{i}",
                                   tag="ps") for i in range(2)]
                    for u in range(N_CORES):
                        t = jtile(0, u)
                        for i in range(2):
                            nc.tensor.matmul(
                                pn[i][:], pbe[t][:, m2 * 128:(m2 + 1) * 128],
                                ekg0[u][:, D + i * 512:D + (i + 1) * 512],
                                start=(u == 0), stop=(u == N_CORES - 1))
                            nc.tensor.matmul(
                                pd[i][:], pbe[t][:, m2 * 128:(m2 + 1) * 128],
                                ekg0[u][:, i * 512:(i + 1) * 512],
                                start=(u == 0), stop=(u == N_CORES - 1))
                    sp = pbp.tile([128, 4 * 512], BF16, name=f"sp{m2}",
                                  tag=f"sp{m2}")
                    for i in range(2):
                        nc.scalar.copy(sp[:, i * 512:(i + 1) * 512], pn[i][:])
                        nc.scalar.copy(
                            sp[:, D + i * 512:D + (i + 1) * 512], pd[i][:])
                    spill[m2] = sp

                # odd pass: chunk-1 partials, merge, y, transpose, out proj
                ekg1 = load_ekg(1)
                for m2 in range(NG):
                    pn = [psp.tile([128, 512], F32, name=f"pnO{m2}{i}",
                                   tag="ps") for i in range(2)]
                    pd = [psp.tile([128, 512], F32, name=f"pdO{m2}{i}",
                                   tag="ps") for i in range(2)]
                    for u in range(N_CORES):
                        t = jtile(1, u)
                        for i in range(2):
                            nc.tensor.matmul(
                                pn[i][:], pbe[t][:, m2 * 128:(m2 + 1) * 128],
                                ekg1[u][:, D + i * 512:D + (i + 1) * 512],
                                start=(u == 0), stop=(u == N_CORES - 1))
                            nc.tensor.matmul(
                                pd[i][:], pbe[t][:, m2 * 128:(m2 + 1) * 128],
                                ekg1[u][:, i * 512:(i + 1) * 512],
                                start=(u == 0), stop=(u == N_CORES - 1))
                    sp = spill[m2]
                    y = pbp.tile([128, D], BF16, name=f"y{m2}", tag="y", bufs=3)
                    for i in range(2):
                        tn = pbp.tile([128, 512], F32, name=f"tn{m2}{i}",
                                      tag="tn", bufs=3)
                        td = pbp.tile([128, 512], F32, name=f"td{m2}{i}",
                                      tag="td", bufs=3)
                        rec = pbp.tile([128, 512], F32, name=f"rc{m2}{i}",
                                       tag="rc", bufs=3)
                        nc.vector.tensor_add(
                            tn[:], pn[i][:], sp[:, i * 512:(i + 1) * 512])
                        nc.vector.tensor_add(
                            td[:], pd[i][:], sp[:, D + i * 512:D + (i + 1) * 512])
                        nc.vector.reciprocal_approx_fast(rec[:], td[:])
                        nc.vector.tensor_mul(tn[:], tn[:], rec[:])
                        nc.vector.tensor_mul(
                            y[:, i * 512:(i + 1) * 512], tn[:],
                            sq_t[m2][:, i * 512:(i + 1) * 512])
                    # y [i, d] -> yT [d, i] via PE transpose, 128x128 blocks
                    yT = [pbp.tile([128, 128], BF16, name=f"yT{m2}_{k}",
                                   tag=f"yT{k}", bufs=2) for k in range(KT)]
                    for k in range(KT):
                        pt = pstr.tile([128, 128], BF16, name=f"pt{m2}{k}",
                                       tag="tr")
                        nc.tensor.transpose(
                            pt[:], y[:, k * 128:(k + 1) * 128], ident[:])
                        nc.vector.tensor_copy(yT[k][:], pt[:])

                    po = [psp.tile([128, 512], F32, name=f"po{m2}_{n}", tag="ps")
                          for n in range(2)]
                    for k in range(KT):
                        for n in range(2):
                            nc.tensor.matmul(
                                po[n][:], yT[k][:],
                                wout[k][:, n * 512:(n + 1) * 512],
                                start=(k == 0),
                                stop=(k == KT - 1 and not with_out_bias))
                    if with_out_bias:
                        for n in range(2):
                            nc.tensor.matmul(
                                po[n][:], ones1[:], bout[:, n * 512:(n + 1) * 512],
                                start=False, stop=True)
                    for n in range(2):
                        osb = pbp.tile([128, 512], F32, name=f"osb{m2}_{n}",
                                       tag="osb", bufs=4)
                        nc.scalar.copy(osb[:], po[n][:])
                        nc.sync.dma_start(
                            out_d[m2 * 128:(m2 + 1) * 128,
                                  n * 512:(n + 1) * 512], osb[:])

    nc.compile()
    return nc


def _prep_inputs(data, W_qkv, b_qkv, pos_bias_param, W_out, b_out):
    bf = ml_dtypes.bfloat16
    data = np.asarray(data, np.float32)
    W_qkv = np.asarray(W_qkv, np.float32)
    b_qkv = np.asarray(b_qkv, np.float32)
    pos_bias_param = np.asarray(pos_bias_param, np.float32)
    W_out = np.asarray(W_out, np.float32)
    b_out = np.asarray(b_out, np.float32)

    with_qkv_bias = bool(np.any(b_qkv))
    with_out_bias = bool(np.any(b_out))

    wq = np.ascontiguousarray(W_qkv[:, :D]).astype(bf)
    wkv = np.ascontiguousarray(W_qkv[:, D:]).astype(bf)
    wout = W_out.astype(bf)
    pbT = np.ascontiguousarray(pos_bias_param.T)  # [j, i]

    p = np.arange(128)
    in_maps = []
    for r in range(N_CORES):
        b, h = r // 2, r % 2
        isl = slice(h * TOK, (h + 1) * TOK)
        dT = np.ascontiguousarray(
            data[isl, b, :].T).astype(bf)                    # [d_in, tok]
        pbT_c = np.ascontiguousarray(pbT[:, isl]).astype(bf)  # [j, i_loc]
        # gidx col = x*16 + half*8 + u': rows of j-tile u' of chunk x
        gidx = np.zeros((128, 32), np.int32)
        for x in range(2):
            for half in range(2):
                for u in range(8):
                    hp, uu = u // 4, u % 4
                    base = (2 * b + hp) * 1024 + half * 512 + uu * 128
                    gidx[:, x * 16 + half * 8 + u] = base + p
        m = {"dataT": dT, "wq": wq, "wkv": wkv, "pbT": pbT_c, "wout": wout,
             "gidx": gidx}
        if with_qkv_bias:
            m["bq"] = np.ascontiguousarray(b_qkv[:D]).reshape(1, D).astype(bf)
            m["bkv"] = np.ascontiguousarray(b_qkv[D:]).reshape(1, 2 * D).astype(bf)
        if with_out_bias:
            m["bout"] = b_out.reshape(1, D).astype(bf)
        in_maps.append(m)
    return in_maps, with_qkv_bias, with_out_bias


def run(data, W_qkv, b_qkv, pos_bias_param, W_out, b_out, **spmd_kwargs):
    in_maps, wb, ob = _prep_inputs(data, W_qkv, b_qkv, pos_bias_param, W_out, b_out)
    key = (wb, ob)
    if key not in _cache:
        _cache[key] = build(wb, ob)
    nc = _cache[key]
    res = run_bass_kernel_spmd(nc, in_maps, core_ids=list(range(N_CORES)),
                               **spmd_kwargs)
    out = np.empty((T, B, D), np.float32)
    for r in range(N_CORES):
        b, h = r // 2, r % 2
        out[h * TOK:(h + 1) * TOK, b, :] = res.results[r]["out"]
    return out, res


def kernel(data, W_qkv, b_qkv, pos_bias_param, W_out, b_out):
    out, _ = run(data, W_qkv, b_qkv, pos_bias_param, W_out, b_out)
    return out
